# revision 7
# baseline (speedup 1.0000x reference)
"""Bass/Trainium2 kernel for the DNC (nn_DNCWrapper) problem.

Self-contained: accepts FULL unsharded inputs (as produced by
setup_inputs()), shards the batch across 8 NeuronCores (64 rows each),
runs the T=512 recurrent scan on-device, and returns the full (512, 10)
output (last-timestep DNC output).

Design notes
------------
- Data-parallel over batch: identical SPMD program on 8 cores, each core
  owns a (64, T, 10) slice of x.  All parameters replicated.
- LSTM controller is computed feature-major (gate features on the 128
  partitions, batch on the free dim).  Gates are reordered [i, f, o, g]
  host-side so one sigmoid covers i,f,o.  b0 is folded in through a
  constant ones-row of the LSTM0 input tile.
- The 88-wide interface vector is computed batch-major directly
  (xi = h1.T @ W_if.T) so the DNC memory phase (inherently batch-major,
  64 partitions) needs no transpose of xi.
- ScalarE activations are restricted to ONE activation table
  (sigmoid_and_others: Sigmoid/Tanh/Copy/Identity/Square) because an
  in-loop activation-table switch costs ~1.4us.  Therefore:
    * softmax exp:   exp(x) = sig(x) / (1 - sig(x))   for x = l - max <= 0
    * softplus(z) = -ln(sig(-z)), with ln computed on VectorE via
      float bit-manipulation (exponent/mantissa split + deg-5 poly)
    * content denominator sqrt(kn2*mn2) via fast-inverse-sqrt bit hack
      + 2 Newton iterations on VectorE (then sqrt(d) = d * rsqrt(d),
      which is exactly 0 for never-written cells, matching the
      reference's  kn*mn + 1e-6  denominator).
- The usage->allocation sort is done WITHOUT argsort: with u in (64,10),
  alloc[j] = (1-u_j) * prod_i { u_i if (u_i<u_j or (u_i==u_j and i<j)) else 1 }
  which exactly matches a stable ascending argsort's exclusive cumprod
  (ties broken by index).  Implemented with pairwise stride-0-broadcast
  compares on (64,10,10) views and a product-reduce.
- ||mem_n||^2 is computed once per step after the memory write and
  cached for the next step's write-content lookup.
- rv is transposed to feature-major once per step via the PE transpose,
  landing directly in the LSTM0 input tile.
"""

import os
import sys

import numpy as np

for _p in ("/opt/trn_rl_repo",):
    if _p not in sys.path and os.path.isdir(_p):
        sys.path.insert(0, _p)

import concourse.bass as bass
import concourse.bacc as bacc
import concourse.mybir as mybir
from concourse import tile
from concourse.bass_utils import run_bass_kernel_spmd

F32 = mybir.dt.float32
I32 = mybir.dt.int32
OP = mybir.AluOpType
AF = mybir.ActivationFunctionType
AX = mybir.AxisListType

# problem constants
NCORES = 8
B = 512
T = 512
IN = 10
CELL = 20
NCELLS = 10
HID = 128
IFACE = 88
BL = B // NCORES  # 64 batch rows per core

LN2 = float(np.log(2.0))
RSQRT_MAGIC = 0x5F3759DF

# minimax-ish (least squares, fine at 1e-6 level) fit of log2(1+x) on [0,1]
_xfit = np.linspace(0, 1, 4001)
_LOG2C = np.polyfit(_xfit, np.log2(1.0 + _xfit), 6)[::-1]  # c0..c6


def _gate_perm():
    # torch gate order [i, f, g, o] -> ours [i, f, o, g]
    return np.concatenate(
        [np.arange(0, 128), np.arange(128, 256), np.arange(384, 512), np.arange(256, 384)]
    )


def _iface_perm():
    # original xi rows: rk 0:20, rs 20, wk 21:41, ws 41, er 42:62, wv 62:82,
    # fg 82, ga 83, gw 84, modes 85:88
    # new: rk 0:20, wk 20:40, wv 40:60, rs 60, ws 61, er 62:82, fg 82,
    # ga 83, gw 84, modes 85:88
    return np.concatenate(
        [
            np.arange(0, 20),
            np.arange(21, 41),
            np.arange(62, 82),
            [20],
            [41],
            np.arange(42, 62),
            [82], [83], [84],
            np.arange(85, 88),
        ]
    )


def build_program(t_steps, b0_nonzero=False, b1_nonzero=False, bif_nonzero=False, unroll_mode="full", max_unroll=8):
    nc = bacc.Bacc()

    xT_d = nc.dram_tensor("xT", [IN, t_steps * BL], F32, kind="ExternalInput")
    W0rvT_d = nc.dram_tensor("W0rvT", [CELL, 512], F32, kind="ExternalInput")
    W0xT_d = nc.dram_tensor("W0xT", [IN, 512], F32, kind="ExternalInput")
    Whh0T_d = nc.dram_tensor("Whh0T", [128, 512], F32, kind="ExternalInput")
    Wih1T_d = nc.dram_tensor("Wih1T", [128, 512], F32, kind="ExternalInput")
    Whh1T_d = nc.dram_tensor("Whh1T", [128, 512], F32, kind="ExternalInput")
    WifT_d = nc.dram_tensor("WifT", [128, IFACE], F32, kind="ExternalInput")
    WoutTh_d = nc.dram_tensor("WoutTh", [128, 10], F32, kind="ExternalInput")
    WoutTr_d = nc.dram_tensor("WoutTr", [CELL, 10], F32, kind="ExternalInput")
    IDXLT_d = nc.dram_tensor("IDXLT", [BL, 100], F32, kind="ExternalInput")
    OFFD_d = nc.dram_tensor("OFFD", [BL, 100], F32, kind="ExternalInput")
    IDENT_d = nc.dram_tensor("IDENT", [128, 128], F32, kind="ExternalInput")
    if b0_nonzero:
        b0r_d = nc.dram_tensor("b0row", [1, 512], F32, kind="ExternalInput")
    if b1_nonzero:
        b1r_d = nc.dram_tensor("b1row", [1, 512], F32, kind="ExternalInput")
    if bif_nonzero:
        bifr_d = nc.dram_tensor("bifrow", [1, IFACE], F32, kind="ExternalInput")
    y_d = nc.dram_tensor("y", [BL, 10], F32, kind="ExternalOutput")

    V = nc.vector
    S = nc.scalar
    G = nc.gpsimd
    PE = nc.tensor

    with tile.TileContext(nc) as tc:
        with (
            tc.tile_pool(name="w", bufs=1) as wp,
            tc.tile_pool(name="st", bufs=1) as sp,
            tc.tile_pool(name="scr", bufs=2) as scr,
            tc.tile_pool(name="ps", bufs=1, space=bass.MemorySpace.PSUM) as pp,
        ):
            # ---- weights / constants ----
            W0rvT = wp.tile([CELL, 512], F32)
            W0xT = wp.tile([IN, 512], F32)
            Whh0T = wp.tile([128, 512], F32)
            Wih1T = wp.tile([128, 512], F32)
            Whh1T = wp.tile([128, 512], F32)
            WifT = wp.tile([128, IFACE], F32)
            WoutTh = wp.tile([128, 10], F32)
            WoutTr = wp.tile([CELL, 10], F32)
            IDXLT = wp.tile([BL, 100], F32)
            OFFD = wp.tile([BL, 100], F32)
            IDENT = wp.tile([128, 128], F32)
            ONES10 = wp.tile([BL, 10], F32)
            ONES100 = wp.tile([BL, 100], F32)
            ONESROW = wp.tile([1, BL], F32)
            Xs = wp.tile([IN, t_steps * BL], F32)

            nc.sync.dma_start(W0rvT[:], W0rvT_d[:])
            nc.sync.dma_start(W0xT[:], W0xT_d[:])
            nc.sync.dma_start(Whh0T[:], Whh0T_d[:])
            nc.sync.dma_start(Wih1T[:], Wih1T_d[:])
            nc.sync.dma_start(Whh1T[:], Whh1T_d[:])
            nc.sync.dma_start(WifT[:], WifT_d[:])
            nc.sync.dma_start(WoutTh[:], WoutTh_d[:])
            nc.sync.dma_start(WoutTr[:], WoutTr_d[:])
            nc.sync.dma_start(IDXLT[:], IDXLT_d[:])
            nc.sync.dma_start(OFFD[:], OFFD_d[:])
            nc.sync.dma_start(IDENT[:], IDENT_d[:])
            nc.sync.dma_start(Xs[:], xT_d[:])
            if b0_nonzero:
                b0row = wp.tile([1, 512], F32)
                nc.sync.dma_start(b0row[:], b0r_d[:])
            if b1_nonzero:
                b1row = wp.tile([1, 512], F32)
                nc.sync.dma_start(b1row[:], b1r_d[:])
            if bif_nonzero:
                bifrow = wp.tile([1, IFACE], F32)
                nc.sync.dma_start(bifrow[:], bifr_d[:])
            V.memset(ONES10[:], 1.0)
            V.memset(ONESROW[:], 1.0)
            V.memset(ONES100[:], 1.0)

            # ---- state ----
            rvT = sp.tile([CELL, BL], F32)
            h0T = sp.tile([128, BL], F32)
            c0 = sp.tile([128, BL], F32)
            h1T = sp.tile([128, BL], F32)
            c1 = sp.tile([128, BL], F32)
            mem = sp.tile([BL, 200], F32)
            nusage = sp.tile([BL, 10], F32)  # holds -usage
            prec = sp.tile([BL, 10], F32)
            link = sp.tile([BL, 100], F32)
            rw = sp.tile([BL, 10], F32)
            ww = sp.tile([BL, 10], F32)
            mn2c = sp.tile([BL, 10], F32)  # cached ||mem_n||^2
            xiT = sp.tile([BL, IFACE], F32)

            G.memset(rvT[:], 0.0)
            G.memset(h0T[:], 0.0)
            G.memset(c0[:], 0.0)
            G.memset(h1T[:], 0.0)
            G.memset(c1[:], 0.0)
            G.memset(mem[:], 0.0)
            G.memset(nusage[:], 0.0)
            G.memset(prec[:], 0.0)
            G.memset(link[:], 0.0)
            G.memset(rw[:], 0.0)
            G.memset(ww[:], 0.0)
            G.memset(mn2c[:], 0.0)

            # persistent psum tiles
            ps_g0 = pp.tile([128, 256], F32)
            ps_g1 = pp.tile([128, 256], F32)
            ps_xi = pp.tile([BL, IFACE], F32)
            ps_rvT = pp.tile([CELL, BL], F32)
            ps_y = pp.tile([BL, 10], F32)

            def bc(ap, axis, shape):
                return ap.unsqueeze(axis).broadcast_to(shape)

            def v3(t_ap, n, w):
                return t_ap.rearrange("p (n w) -> p n w", n=n, w=w)

            def v3t(t_ap, n, w):
                return t_ap.rearrange("p (n w) -> p w n", n=n, w=w)

            def lstm(ps_g, pairs, cstate, hstate):
                for g in range(4):
                    for k, (Wm, rhs) in enumerate(pairs):
                        PE.matmul(
                            ps_g[:, g * BL : (g + 1) * BL],
                            Wm[:, g * 128 : (g + 1) * 128],
                            rhs,
                            start=(k == 0),
                            stop=(k == len(pairs) - 1),
                        )
                sig = scr.tile([128, 3 * BL], F32, tag="sig")
                tg = scr.tile([128, BL], F32, tag="tg")
                tcc = scr.tile([128, BL], F32, tag="tcc")
                ig = scr.tile([128, BL], F32, tag="ig")
                S.activation(sig[:], ps_g[:, 0 : 3 * BL], AF.Sigmoid)
                S.activation(tg[:], ps_g[:, 3 * BL : 4 * BL], AF.Tanh)
                V.tensor_tensor(ig[:], sig[:, 0:BL], tg[:], OP.mult)
                V.tensor_tensor(cstate[:], cstate[:], sig[:, BL : 2 * BL], OP.mult)
                V.tensor_add(cstate[:], cstate[:], ig[:])
                S.activation(tcc[:], cstate[:], AF.Tanh)
                V.tensor_tensor(hstate[:], sig[:, 2 * BL : 3 * BL], tcc[:], OP.mult)

            def softmax_ratio(logits_ap, k, e_tile, rsum_tile, name):
                """e = exp(l - max) via sigmoid ratio; rsum = 1/sum(e)."""
                negmx = scr.tile([BL, 1], F32, tag=f"nm_{name}")
                spos = scr.tile([BL, k], F32, tag=f"sp_{name}")
                sneg = scr.tile([BL, k], F32, tag=f"sn_{name}")
                ssum = scr.tile([BL, 1], F32, tag=f"ss_{name}")
                V.tensor_reduce(negmx[:], logits_ap, axis=AX.X, op=OP.max, negate=True)
                S.activation(spos[:], logits_ap, AF.Sigmoid, bias=negmx[:])
                V.tensor_scalar(sneg[:], spos[:], -1.0, 1.0, op0=OP.mult, op1=OP.add)
                V.reciprocal(sneg[:], sneg[:])
                V.tensor_tensor(e_tile[:], spos[:], sneg[:], OP.mult)
                V.tensor_reduce(ssum[:], e_tile[:], axis=AX.X, op=OP.add)
                V.reciprocal(rsum_tile[:], ssum[:])

            def rsqrt_block(d_tile, k, name):
                """returns tile y ~= 1/sqrt(d) (elementwise, d>=0; d=0 -> huge)."""
                y = scr.tile([BL, k], F32, tag=f"rsq_{name}")
                t1 = scr.tile([BL, k], F32, tag=f"rsqa_{name}")
                di = d_tile[:].bitcast(I32)
                yi = y[:].bitcast(I32)
                t1i = t1[:].bitcast(I32)
                V.tensor_scalar(t1i, di, 1, None, op0=OP.arith_shift_right)
                V.tensor_scalar(yi, t1i, -1, RSQRT_MAGIC, op0=OP.mult, op1=OP.add)
                for _ in range(2):
                    V.tensor_tensor(t1[:], y[:], y[:], OP.mult)
                    V.tensor_tensor(t1[:], t1[:], d_tile[:], OP.mult)
                    V.tensor_scalar(t1[:], t1[:], -0.5, 1.5, op0=OP.mult, op1=OP.add)
                    V.tensor_tensor(y[:], y[:], t1[:], OP.mult)
                return y

            def content_den(mn2_ap, kn2_ap, name):
                """rden = 1 / (sqrt(mn2*kn2) + 1e-6)   (shape (BL,10))"""
                d = scr.tile([BL, 10], F32, tag=f"d_{name}")
                V.tensor_scalar_mul(d[:], mn2_ap, kn2_ap)  # kn2 per-partition scalar
                V.tensor_scalar_max(d[:], d[:], 1e-30)
                y = rsqrt_block(d, 10, name)
                V.tensor_tensor(d[:], d[:], y[:], OP.mult)  # sqrt(d) = d * rsqrt(d)
                V.tensor_scalar_add(d[:], d[:], 1e-6)
                V.reciprocal(d[:], d[:])
                return d

            def ln_neg_block(w_tile, k, out_ap, name):
                """out = -ln(w) for w in (0,1], via exponent/mantissa split."""
                ei = scr.tile([BL, k], I32, tag=f"ei_{name}")
                mi = scr.tile([BL, k], I32, tag=f"mi_{name}")
                ef = scr.tile([BL, k], F32, tag=f"ef_{name}")
                mf = scr.tile([BL, k], F32, tag=f"mf_{name}")
                acc = scr.tile([BL, k], F32, tag=f"ac_{name}")
                wi = w_tile[:].bitcast(I32)
                V.tensor_scalar(ei[:], wi, 23, None, op0=OP.logical_shift_right)
                V.tensor_scalar(mi[:], wi, 0x7FFFFF, None, op0=OP.bitwise_and)
                V.tensor_copy(ef[:], ei[:])  # int -> float
                V.tensor_copy(mf[:], mi[:])
                V.tensor_scalar(mf[:], mf[:], float(2.0 ** -23), None, op0=OP.mult)
                c = [float(v) for v in _LOG2C]  # c0..c6
                # Horner: acc = ((c6*m + c5)*m + ...) + c0
                V.tensor_scalar(acc[:], mf[:], c[6], c[5], op0=OP.mult, op1=OP.add)
                for idx in (4, 3, 2, 1, 0):
                    V.tensor_tensor(acc[:], acc[:], mf[:], OP.mult)
                    V.tensor_scalar_add(acc[:], acc[:], c[idx])
                # ln(w) = ln2 * (ef - 127 + acc);  out = -ln(w)
                V.tensor_add(acc[:], acc[:], ef[:])
                V.tensor_scalar(out_ap, acc[:], -LN2, 127.0 * LN2, op0=OP.mult, op1=OP.add)

            def body(tv, dynamic):
                if dynamic:
                    xsl = Xs[:, bass.ds(tv * BL, BL)]
                else:
                    xsl = Xs[:, tv * BL : (tv + 1) * BL]

                # ---- LSTM0 / LSTM1 ----
                pairs0 = [(W0rvT, rvT[:]), (W0xT, xsl), (Whh0T, h0T[:])]
                if b0_nonzero:
                    pairs0.append((b0row, ONESROW[:]))
                lstm(ps_g0, pairs0, c0, h0T)
                pairs1 = [(Wih1T, h0T[:]), (Whh1T, h1T[:])]
                if b1_nonzero:
                    pairs1.append((b1row, ONESROW[:]))
                lstm(ps_g1, pairs1, c1, h1T)

                # ---- interface: xi = ctrl @ W_if.T (batch-major) ----
                PE.matmul(ps_xi[:], h1T[:], WifT[:], start=True, stop=not bif_nonzero)
                if bif_nonzero:
                    PE.matmul(ps_xi[:], ONESROW[:], bifrow[:], start=False, stop=True)

                S.activation(xiT[:, 0:60], ps_xi[:, 0:60], AF.Tanh)
                S.activation(xiT[:, 62:85], ps_xi[:, 62:85], AF.Sigmoid)
                # softplus(z) = -ln(sigmoid(-z)) for rs, ws
                wsp = scr.tile([BL, 2], F32, tag="wsp")
                S.activation(wsp[:], ps_xi[:, 60:62], AF.Sigmoid, scale=-1.0)
                ln_neg_block(wsp, 2, xiT[:, 60:62], "sp")
                # modes softmax
                e3 = scr.tile([BL, 3], F32, tag="e3")
                rs3 = scr.tile([BL, 3], F32, tag="rs3")
                rsum3 = scr.tile([BL, 1], F32, tag="rsum3")
                softmax_ratio(ps_xi[:, 85:88], 3, e3, rsum3, "m")
                modes = scr.tile([BL, 3], F32, tag="modes")
                V.tensor_scalar_mul(modes[:], e3[:], rsum3[:])

                rk = xiT[:, 0:20]
                wk = xiT[:, 20:40]
                wv = xiT[:, 40:60]
                rs = xiT[:, 60:61]
                ws = xiT[:, 61:62]
                er = xiT[:, 62:82]
                fg = xiT[:, 82:83]
                ga = xiT[:, 83:84]
                gw = xiT[:, 84:85]

                # ---- usage update ----
                npsi = scr.tile([BL, 10], F32, tag="npsi")
                t10a = scr.tile([BL, 10], F32, tag="t10a")
                t10b = scr.tile([BL, 10], F32, tag="t10b")
                u = scr.tile([BL, 10], F32, tag="u")
                V.scalar_tensor_tensor(npsi[:], rw[:], fg, ONES10[:], op0=OP.mult, op1=OP.subtract)
                V.scalar_tensor_tensor(t10a[:], nusage[:], 1.0, ww[:], op0=OP.add, op1=OP.mult)
                V.tensor_sub(t10b[:], t10a[:], nusage[:])
                V.tensor_tensor(nusage[:], t10b[:], npsi[:], OP.mult)
                V.tensor_scalar(u[:], nusage[:], -(1.0 - 1e-6), 1e-6, op0=OP.mult, op1=OP.add)

                # ---- allocation weights (sort-free) ----
                t100a = scr.tile([BL, 100], F32, tag="t100a")
                t100b = scr.tile([BL, 100], F32, tag="t100b")
                excl = scr.tile([BL, 10], F32, tag="excl")
                ou = scr.tile([BL, 10], F32, tag="ou")
                alloc = scr.tile([BL, 10], F32, tag="alloc")
                uT = bc(u[:], 1, [BL, 10, 10])  # [b,j,i] = u_i
                uR = bc(u[:], 2, [BL, 10, 10])  # [b,j,i] = u_j
                a3a = v3(t100a[:], 10, 10)
                a3b = v3(t100b[:], 10, 10)
                V.tensor_tensor(a3a, uT, uR, OP.is_lt)
                V.tensor_tensor(a3b, uT, uR, OP.is_equal)
                V.tensor_tensor(t100b[:], t100b[:], IDXLT[:], OP.mult)
                V.tensor_add(t100a[:], t100a[:], t100b[:])  # mask
                V.scalar_tensor_tensor(a3a, uT, -1.0, a3a, op0=OP.add, op1=OP.mult)
                V.tensor_scalar_add(t100a[:], t100a[:], 1.0)
                # product over i (last axis) via pairwise multiply tree
                V.tensor_tensor(a3a[:, :, 0:5], a3a[:, :, 0:5], a3a[:, :, 5:10], OP.mult)
                V.tensor_tensor(a3a[:, :, 0:2], a3a[:, :, 0:2], a3a[:, :, 2:4], OP.mult)
                V.tensor_tensor(a3a[:, :, 0:1], a3a[:, :, 0:1], a3a[:, :, 1:2], OP.mult)
                V.tensor_tensor(excl[:].unsqueeze(2), a3a[:, :, 0:1], a3a[:, :, 4:5], OP.mult)
                V.tensor_scalar(ou[:], u[:], -1.0, 1.0, op0=OP.mult, op1=OP.add)
                V.tensor_tensor(alloc[:], ou[:], excl[:], OP.mult)

                # ---- write content lookup (pre-write mem, cached mn2c) ----
                t200 = scr.tile([BL, 200], F32, tag="t200")
                dotw = scr.tile([BL, 10], F32, tag="dotw")
                junk20 = scr.tile([BL, 20], F32, tag="junk20")
                kn2w = scr.tile([BL, 1], F32, tag="kn2w")
                logw = scr.tile([BL, 10], F32, tag="logw")
                ew = scr.tile([BL, 10], F32, tag="ew")
                rsumw = scr.tile([BL, 1], F32, tag="rsumw")
                V.tensor_tensor(v3(t200[:], 10, 20), v3(mem[:], 10, 20), bc(wk, 1, [BL, 10, 20]), OP.mult)
                V.tensor_reduce(dotw[:], v3(t200[:], 10, 20), axis=AX.X, op=OP.add)
                S.activation(junk20[:], wk, AF.Square, accum_out=kn2w[:])
                rdenw = content_den(mn2c[:], kn2w[:], "w")
                V.scalar_tensor_tensor(logw[:], dotw[:], ws, rdenw[:], op0=OP.mult, op1=OP.mult)
                softmax_ratio(logw[:], 10, ew, rsumw, "w")

                # ---- ww = gw*(ga*alloc + (1-ga)*cw) ----
                sa = scr.tile([BL, 1], F32, tag="sa")
                sc1 = scr.tile([BL, 1], F32, tag="sc1")
                sc3 = scr.tile([BL, 1], F32, tag="sc3")
                wwt = scr.tile([BL, 10], F32, tag="wwt")
                V.tensor_tensor(sa[:], gw, ga, OP.mult)
                V.tensor_scalar(sc1[:], ga, -1.0, 1.0, op0=OP.mult, op1=OP.add)
                V.tensor_tensor(sc1[:], sc1[:], gw, OP.mult)
                V.tensor_tensor(sc3[:], sc1[:], rsumw[:], OP.mult)
                V.tensor_scalar_mul(wwt[:], alloc[:], sa[:])
                V.scalar_tensor_tensor(ww[:], ew[:], sc3[:], wwt[:], op0=OP.mult, op1=OP.add)

                # ---- memory write: mem -= ww_n * (er . mem - wv) ----
                V.tensor_tensor(v3(t200[:], 10, 20), v3(mem[:], 10, 20), bc(er, 1, [BL, 10, 20]), OP.mult)
                V.tensor_sub(v3(t200[:], 10, 20), v3(t200[:], 10, 20), bc(wv, 1, [BL, 10, 20]))
                V.tensor_tensor(v3(t200[:], 10, 20), v3(t200[:], 10, 20), bc(ww[:], 2, [BL, 10, 20]), OP.mult)
                V.tensor_sub(mem[:], mem[:], t200[:])

                # ---- mn2 cache refresh (post-write) ----
                t200b = scr.tile([BL, 200], F32, tag="t200b")
                V.tensor_tensor(t200b[:], mem[:], mem[:], OP.mult)
                V.tensor_reduce(mn2c[:], v3(t200b[:], 10, 20), axis=AX.X, op=OP.add)

                # ---- link update ----
                t100L = scr.tile([BL, 100], F32, tag="t100L")
                t100M = scr.tile([BL, 100], F32, tag="t100M")
                V.tensor_tensor(v3(t100L[:], 10, 10), bc(ww[:], 2, [BL, 10, 10]), bc(ww[:], 1, [BL, 10, 10]), OP.add)
                V.scalar_tensor_tensor(t100L[:], t100L[:], -1.0, ONES100[:], op0=OP.mult, op1=OP.add)
                V.tensor_tensor(link[:], link[:], t100L[:], OP.mult)
                V.tensor_tensor(v3(t100M[:], 10, 10), bc(ww[:], 2, [BL, 10, 10]), bc(prec[:], 1, [BL, 10, 10]), OP.mult)
                V.tensor_add(link[:], link[:], t100M[:])
                V.tensor_tensor(link[:], link[:], OFFD[:], OP.mult)

                # ---- precedence ----
                nws = scr.tile([BL, 1], F32, tag="nws")
                t10c = scr.tile([BL, 10], F32, tag="t10c")
                V.tensor_reduce(nws[:], ww[:], axis=AX.X, op=OP.add, negate=True)
                V.scalar_tensor_tensor(t10c[:], prec[:], nws[:], prec[:], op0=OP.mult, op1=OP.add)
                V.tensor_add(prec[:], t10c[:], ww[:])

                # ---- forward / backward read weights ----
                t100f = scr.tile([BL, 100], F32, tag="t100f")
                t100g = scr.tile([BL, 100], F32, tag="t100g")
                fwd = scr.tile([BL, 10], F32, tag="fwd")
                bwd = scr.tile([BL, 10], F32, tag="bwd")
                V.tensor_tensor(v3(t100f[:], 10, 10), v3(link[:], 10, 10), bc(rw[:], 1, [BL, 10, 10]), OP.mult)
                V.tensor_reduce(fwd[:], v3(t100f[:], 10, 10), axis=AX.X, op=OP.add)
                V.tensor_tensor(v3(t100g[:], 10, 10), v3(link[:], 10, 10), bc(rw[:], 2, [BL, 10, 10]), OP.mult)
                V.tensor_reduce(bwd[:], v3t(t100g[:], 10, 10), axis=AX.X, op=OP.add)

                # ---- read content lookup (post-write mem) ----
                t200c = scr.tile([BL, 200], F32, tag="t200c")
                dotr = scr.tile([BL, 10], F32, tag="dotr")
                kn2r = scr.tile([BL, 1], F32, tag="kn2r")
                junk20b = scr.tile([BL, 20], F32, tag="junk20b")
                logr = scr.tile([BL, 10], F32, tag="logr")
                ecr = scr.tile([BL, 10], F32, tag="ecr")
                rsumr = scr.tile([BL, 1], F32, tag="rsumr")
                V.tensor_tensor(v3(t200c[:], 10, 20), v3(mem[:], 10, 20), bc(rk, 1, [BL, 10, 20]), OP.mult)
                V.tensor_reduce(dotr[:], v3(t200c[:], 10, 20), axis=AX.X, op=OP.add)
                S.activation(junk20b[:], rk, AF.Square, accum_out=kn2r[:])
                rdenr = content_den(mn2c[:], kn2r[:], "r")
                V.scalar_tensor_tensor(logr[:], dotr[:], rs, rdenr[:], op0=OP.mult, op1=OP.mult)
                softmax_ratio(logr[:], 10, ecr, rsumr, "r")

                # ---- rw = m0*bwd + m1*cr + m2*fwd ----
                m1r = scr.tile([BL, 1], F32, tag="m1r")
                trw = scr.tile([BL, 10], F32, tag="trw")
                V.tensor_tensor(m1r[:], modes[:, 1:2], rsumr[:], OP.mult)
                V.tensor_scalar_mul(trw[:], bwd[:], modes[:, 0:1])
                V.scalar_tensor_tensor(trw[:], ecr[:], m1r[:], trw[:], op0=OP.mult, op1=OP.add)
                V.scalar_tensor_tensor(rw[:], fwd[:], modes[:, 2:3], trw[:], op0=OP.mult, op1=OP.add)

                # ---- rv = rw @ mem, transpose into inp31 rows 0:20 ----
                t200d = scr.tile([BL, 200], F32, tag="t200d")
                rv = scr.tile([BL, 20], F32, tag="rv")
                V.tensor_tensor(v3(t200d[:], 10, 20), v3(mem[:], 10, 20), bc(rw[:], 2, [BL, 10, 20]), OP.mult)
                V.tensor_reduce(rv[:], v3t(t200d[:], 10, 20), axis=AX.X, op=OP.add)
                PE.transpose(ps_rvT[:], rv[:], IDENT[0:BL, 0:BL])
                S.activation(rvT[:], ps_rvT[:], AF.Copy)

            if unroll_mode == "full":
                for t in range(t_steps):
                    body(t, dynamic=False)
            else:
                tc.For_i_unrolled(0, t_steps, 1, lambda iv: body(iv, True), max_unroll=max_unroll)

            # ---- output: y = [ctrl, rv] @ W_out.T ----
            y_sb = sp.tile([BL, 10], F32)
            PE.matmul(ps_y[:], h1T[:], WoutTh[:], start=True, stop=False)
            PE.matmul(ps_y[:], rvT[:], WoutTr[:], start=False, stop=True)
            S.activation(y_sb[:], ps_y[:], AF.Copy)
            nc.sync.dma_start(y_d[:], y_sb[:])

    nc.finalize()
    return nc


def prep_inputs(x, W_ih0, W_hh0, b0, W_ih1, W_hh1, b1, W_if, b_if, W_out, b_out, t_steps=T):
    gp = _gate_perm()
    ip = _iface_perm()
    W_ih0p = np.asarray(W_ih0, np.float32)[gp]
    W_hh0p = np.asarray(W_hh0, np.float32)[gp]
    b0p = np.asarray(b0, np.float32)[gp]
    W_ih1p = np.asarray(W_ih1, np.float32)[gp]
    W_hh1p = np.asarray(W_hh1, np.float32)[gp]
    b1p = np.asarray(b1, np.float32)[gp]
    W_ifp = np.asarray(W_if, np.float32)[ip]
    b_ifp = np.asarray(b_if, np.float32)[ip]
    W_outn = np.asarray(W_out, np.float32)


    idx = np.arange(10)
    IDXLT = (idx[None, :] < idx[:, None]).astype(np.float32)  # [j,i] = i<j
    IDXLT = np.broadcast_to(IDXLT.reshape(1, 100), (BL, 100)).copy()
    OFFD = (1.0 - np.eye(10, dtype=np.float32)).reshape(1, 100)
    OFFD = np.broadcast_to(OFFD, (BL, 100)).copy()
    IDENT = np.eye(128, dtype=np.float32)

    common = {
        "W0rvT": np.ascontiguousarray(W_ih0p[:, 10:30].T),
        "W0xT": np.ascontiguousarray(W_ih0p[:, 0:10].T),
        "Whh0T": np.ascontiguousarray(W_hh0p.T),
        "Wih1T": np.ascontiguousarray(W_ih1p.T),
        "Whh1T": np.ascontiguousarray(W_hh1p.T),
        "WifT": np.ascontiguousarray(W_ifp.T),
        "WoutTh": np.ascontiguousarray(W_outn[:, 0:128].T),
        "WoutTr": np.ascontiguousarray(W_outn[:, 128:148].T),
        "IDXLT": IDXLT,
        "OFFD": OFFD,
        "IDENT": IDENT,
    }
    b0_nonzero = bool(np.any(b0p != 0))
    b1_nonzero = bool(np.any(b1p != 0))
    bif_nonzero = bool(np.any(b_ifp != 0))
    if b0_nonzero:
        common["b0row"] = b0p.reshape(1, 512).astype(np.float32)
    if b1_nonzero:
        common["b1row"] = b1p.reshape(1, 512).astype(np.float32)
    if bif_nonzero:
        common["bifrow"] = b_ifp.reshape(1, IFACE).astype(np.float32)

    x = np.asarray(x, np.float32)
    per_core = []
    for c in range(NCORES):
        xc = x[c * BL : (c + 1) * BL, :t_steps, :]  # (BL, t, 10)
        xT = np.ascontiguousarray(xc.transpose(2, 1, 0)).reshape(IN, t_steps * BL)
        m = dict(common)
        m["xT"] = xT
        per_core.append(m)
    return per_core, b0_nonzero, b1_nonzero, bif_nonzero


_CACHE = {}


def _get_program(t_steps, b0nz, b1nz, bifnz, unroll_mode="full", max_unroll=8):
    key = (t_steps, b0nz, b1nz, bifnz, unroll_mode, max_unroll)
    if key not in _CACHE:
        _CACHE[key] = build_program(
            t_steps, b0_nonzero=b0nz, b1_nonzero=b1nz, bif_nonzero=bifnz,
            unroll_mode=unroll_mode, max_unroll=max_unroll,
        )
    return _CACHE[key]


def kernel(x, W_ih0, W_hh0, b0, W_ih1, W_hh1, b1, W_if, b_if, W_out, b_out):
    x = np.asarray(x, np.float32)
    assert x.shape == (B, T, IN), x.shape
    per_core, b0nz, b1nz, bifnz = prep_inputs(
        x, W_ih0, W_hh0, b0, W_ih1, W_hh1, b1, W_if, b_if, W_out, b_out, t_steps=T
    )
    nc = _get_program(T, b0nz, b1nz, bifnz)
    res = run_bass_kernel_spmd(nc, per_core, core_ids=list(range(NCORES)))
    ys = [np.asarray(res.results[c]["y"]) for c in range(NCORES)]
    y = np.concatenate(ys, axis=0).astype(np.float32)
    y = y + np.asarray(b_out, np.float32)[None, :]
    return y
